# revision 34
# baseline (speedup 1.0000x reference)
"""Distributed Trainium2 kernel for causal GQA attention with RoPE.

Model: B=2, S=2048, DM=2048, H=16 q-heads, HK=4 kv-heads, D=128.
Sharding over 8 NeuronCores: core c = (batch b=c//4, kv-head kh=c%4).
Each core computes its 4 q-heads / 1 kv-head of one batch end-to-end,
AllGathers attention outputs within its 4-core batch group, and applies
a column slice of Wo, producing out[b][:, kh*512:(kh+1)*512].

v3 notes:
- all inputs host-pre-tiled so every DMA is a single fully-contiguous
  block; rope tables are bf16 per-chunk tiles (chunk 0 first).
- the 1/sqrt(D) score scale is folded into Wk on the host, so every exp
  runs with scale=1 and shares the warm-up exp's scale/bias constant
  (a scale constant would otherwise be DMA'd behind the startup flood).
- PE warm-up matmuls at t=0 open the HAM clock gate during the
  DMA-bound startup; chunk-0 q1..q3 projections interleave into
  quarter-0 attention (one proj group per head-start/head-end slot).
- rope: DVE copies raw PSUM->SBUF bf16 (frees the proj PSUM bank fast),
  bf16 muls with bf16 tables, 64-partition shift via vector-queue DMA.
- causal mask folded into the score matmul (NEG upper-triangle lhsT @
  identity, accumulated before k.T@q) - no DVE mask add.
- PSUM banks: scores 3 (lookahead 2), proj 1, PV 2, den 1, Wo 1.
- tail: quarter 3 gathers per head right after each head's output; Wo
  accumulates in 4 per-token-block PSUM tiles across 4 waves so only
  the last head's 16 matmuls trail the final gather.
"""
import contextlib
import ctypes
import os
import sys
import types
from collections import deque

for _p in ("/opt/trn_rl_repo", "/root/.axon_site/_ro/trn_rl_repo"):
    if os.path.isdir(_p) and _p not in sys.path:
        sys.path.insert(0, _p)

import numpy as np
import ml_dtypes

import concourse.bass as bass
import concourse.mybir as mybir
import concourse.tile as tile
from concourse import bacc
from concourse.bass import ts, ds
from concourse.bass_utils import run_bass_kernel_spmd

BF16 = ml_dtypes.bfloat16
F32 = mybir.dt.float32
BF = mybir.dt.bfloat16

B, S, DM = 2, 2048, 2048
H, HK, D = 16, 4, 128
G = H // HK          # q heads per kv head (= heads per core)
THETA = 10000.0
N_CORES = 8
KT = DM // 128       # 16 K-tiles of the model dim
TOKB = S // 128      # 16 token blocks
TCH = S // 512       # 4 token chunks of 512
HD_CORE = G * D      # 512 output dims of q per core
NEG = -1.0e30
LOOKAHEAD = 2        # score tiles in flight ahead of PV

LAST_EXEC_TIME_NS = None
LAST_RESULTS = None


# ---------------------------------------------------------------- tracing
def _install_ntff_hook():
    """Make run_bass_kernel_spmd(trace=True) work in this container."""
    try:
        from antenv.axon_hooks import get_axon_ntff_profile_hook  # noqa: F401
        return True
    except ImportError:
        pass
    so_path = "/opt/axon/libaxon_pjrt.so"
    if not os.path.exists(so_path):
        return False
    lib = ctypes.CDLL(so_path)
    if not hasattr(lib, "axon_start_nrt_profile"):
        return False
    lib.axon_start_nrt_profile.argtypes = [ctypes.POINTER(ctypes.c_int64), ctypes.c_size_t]
    lib.axon_start_nrt_profile.restype = ctypes.c_int64
    lib.axon_stop_nrt_profile.argtypes = [ctypes.c_char_p]
    lib.axon_stop_nrt_profile.restype = ctypes.c_int64

    @contextlib.contextmanager
    def _hook(output_dir, device_ids):
        import jax
        jax.devices()
        if device_ids:
            ids = (ctypes.c_int64 * len(device_ids))(*device_ids)
            rc = lib.axon_start_nrt_profile(ids, len(device_ids))
        else:
            rc = lib.axon_start_nrt_profile(None, 0)
        if rc != 0:
            raise RuntimeError(f"axon_start_nrt_profile rc={rc}")
        try:
            yield
        finally:
            n = lib.axon_stop_nrt_profile(str(output_dir).encode())
            print(f"profile: {n} file(s) in {output_dir}", file=sys.stderr)

    mod = types.ModuleType("antenv.axon_hooks")
    holder = {"h": _hook}
    mod.set_axon_ntff_profile_hook = lambda h: holder.__setitem__("h", h)
    mod.get_axon_ntff_profile_hook = lambda: holder.get("h")
    sys.modules["antenv.axon_hooks"] = mod
    import antenv
    antenv.axon_hooks = mod
    import concourse.bass_utils as bu
    bu.upload_artifacts = lambda tmpdir: str(tmpdir)
    return True


# ---------------------------------------------------------------- graph
def build_nc():
    nc = bacc.Bacc("TRN2", target_bir_lowering=False, debug=False,
                   num_devices=N_CORES)

    xt = nc.dram_tensor("xt", [TCH, KT, 128, 512], BF,
                        kind="ExternalInput").ap()
    # qkv weights host-paired: two adjacent 128-wide k-tiles per DMA tile
    # (half the dma_start issues, 512B rows instead of 256B)
    wq = nc.dram_tensor("wq", [G, KT // 2, 128, 256], BF,
                        kind="ExternalInput").ap()
    wk = nc.dram_tensor("wk", [KT // 2, 128, 2 * D], BF,
                        kind="ExternalInput").ap()
    wv = nc.dram_tensor("wv", [KT // 2, 128, 2 * D], BF,
                        kind="ExternalInput").ap()
    wo = nc.dram_tensor("wo", [KT, 128, HD_CORE], BF,
                        kind="ExternalInput").ap()
    cosb = nc.dram_tensor("cosb", [TCH, D, 512], BF,
                          kind="ExternalInput").ap()
    sinb = nc.dram_tensor("sinb", [TCH, D, 512], BF,
                          kind="ExternalInput").ap()
    cmut = nc.dram_tensor("cmut", [128, 128], BF, kind="ExternalInput").ap()
    iden = nc.dram_tensor("iden", [128, 128], BF, kind="ExternalInput").ap()
    out = nc.dram_tensor("out", [S, HD_CORE], F32, kind="ExternalOutput").ap()

    groups = [[0, 1, 2, 3], [4, 5, 6, 7]]

    with tile.TileContext(nc) as tc:
        with tc.tile_pool(name="const", bufs=1) as cpool, \
             tc.tile_pool(name="wts", bufs=1) as wpool, \
             tc.tile_pool(name="acts", bufs=1) as apool, \
             tc.tile_pool(name="xin", bufs=64) as xpool, \
             tc.tile_pool(name="work", bufs=2) as work, \
             tc.tile_pool(name="etwork", bufs=7) as etwork, \
             tc.tile_pool(name="ogp", bufs=20) as ogpool, \
             tc.tile_pool(name="stats", bufs=2) as stats, \
             tc.tile_pool(name="bcp", bufs=2) as bcpool, \
             tc.tile_pool(name="pssc", bufs=3, space="PSUM") as ps_sc, \
             tc.tile_pool(name="pspj", bufs=1, space="PSUM") as ps_pj, \
             tc.tile_pool(name="pspv", bufs=2, space="PSUM") as ps_pv, \
             tc.tile_pool(name="psden", bufs=1, space="PSUM") as ps_den, \
             tc.tile_pool(name="pswo", bufs=1, space="PSUM") as ps_wo, \
             tc.tile_pool(name="dram", bufs=1, space="DRAM") as dpool:

            # ---------------- constants (host-built, tiny: land first)
            cmut_sb = cpool.tile([128, 128], BF, tag="cmut", name="cmut")
            nc.scalar.dma_start(out=cmut_sb[:], in_=cmut[:])
            iden_sb = cpool.tile([128, 128], BF, tag="iden", name="iden")
            nc.scalar.dma_start(out=iden_sb[:], in_=iden[:])

            # warm the ACT exp table so the first real exp is fast
            warm_act = cpool.tile([128, 1], F32, tag="warm_act",
                                  name="warm_act")
            nc.gpsimd.memset(warm_act[:], 0.0)
            nc.scalar.activation(out=warm_act[:], in_=warm_act[:],
                                 func=mybir.ActivationFunctionType.Exp)
            ones_sb = cpool.tile([128, 1], BF, tag="ones", name="ones")
            nc.gpsimd.memset(ones_sb[:], 1.0)

            # PE warm-up: ~3.5us of dependency-free matmuls at t=0 so the
            # HAM clock-gate opens to 8/8 during the DMA-bound startup
            warm_rhs = cpool.tile([128, 512], BF, tag="warm_rhs",
                                  name="warm_rhs")
            nc.gpsimd.memset(warm_rhs[:], 0.0)
            warm_ps = ps_wo.tile([128, 512], F32, tag="wo", name="warm_ps")
            for _ in range(8):
                nc.tensor.matmul(warm_ps[:], warm_rhs[:, 0:128],
                                 warm_rhs[:], start=True, stop=True)

            # ---------------- rope tables: per-chunk bf16, chunk 0 first
            cos_t = [None] * TCH
            sin_t = [None] * TCH

            def load_tbl(c):
                ct = cpool.tile([D, 512], BF, tag=f"cos{c}", name=f"cos{c}")
                nc.gpsimd.dma_start(out=ct[:], in_=cosb[c])
                st = cpool.tile([D, 512], BF, tag=f"sin{c}", name=f"sin{c}")
                nc.gpsimd.dma_start(out=st[:], in_=sinb[c])
                cos_t[c], sin_t[c] = ct, st

            load_tbl(0)
            # warm up the collective path early
            warm_in = dpool.tile([128, 4], BF, tag="warm_in", name="warm_in")
            warm_out = dpool.tile([4, 128, 4], BF, tag="warm_out",
                                  name="warm_out")
            nc.gpsimd.dma_start(out=warm_in[:], in_=cosb[0, :, 0:4])
            nc.gpsimd.collective_compute(
                "AllGather", mybir.AluOpType.bypass,
                replica_groups=groups,
                ins=[warm_in.opt()], outs=[warm_out.opt()])
            load_tbl(1)
            load_tbl(2)
            load_tbl(3)

            # ---------------- weights: scalar queue, critical first.
            # wo loads are deferred into quarter 1 ("wol" slots).
            wk_pairs = [wpool.tile([128, 2 * D], BF, tag=f"wk{i}",
                                   name=f"wk{i}") for i in range(KT // 2)]
            wq_pairs = [[wpool.tile([128, 256], BF, tag=f"wq{h}_{i}",
                                    name=f"wq{h}_{i}")
                         for i in range(KT // 2)] for h in range(G)]
            wv_pairs = [wpool.tile([128, 2 * D], BF, tag=f"wv{i}",
                                   name=f"wv{i}") for i in range(KT // 2)]
            wo_sb = [wpool.tile([128, HD_CORE], BF, tag=f"wo{kt}",
                                name=f"wo{kt}") for kt in range(KT)]
            def wk_at(kt):
                return wk_pairs[kt // 2][:, ts(kt % 2, 128)]

            def wq_at(h, kt):
                return wq_pairs[h][kt // 2][:, ts(kt % 2, 128)]

            def wv_at(kt):
                return wv_pairs[kt // 2][:, ts(kt % 2, 128)]

            # Each dma_start costs ~600ns of sequencer time (plus ring-full
            # waits), and queues are strict FIFO: anything behind a long
            # DMA-issue train is delayed by it.  So the issues are split:
            #   scalar: wk + wq0 only (34 issues) - exps must start early
            #   gpsimd: tables, wv, wq1 - before the first rope shifts
            #   sync:   xc0, xc1, wq2, wq3 (xc2/xc3 deferred via enq)
            for i in range(KT // 2):
                nc.scalar.dma_start(out=wk_pairs[i][:], in_=wk[i])
            for i in range(KT // 2):
                nc.scalar.dma_start(out=wq_pairs[0][i][:], in_=wq[0, i])
            for i in range(KT // 2):
                nc.gpsimd.dma_start(out=wv_pairs[i][:], in_=wv[i])
            for i in range(KT // 2):
                nc.gpsimd.dma_start(out=wq_pairs[1][i][:], in_=wq[1, i])

            xc_state = [[xpool.tile([128, 512], BF, tag="xc", name="xc")
                         for _ in range(KT)] for _ in range(TCH)]

            def load_xc(c):
                for kt in range(KT):
                    nc.sync.dma_start(out=xc_state[c][kt][:], in_=xt[c, kt])

            # chunks 0/1 stream now; 2/3 deferred into quarter 0 so they
            # don't steal HBM bandwidth from the critical startup loads
            load_xc(0)
            load_xc(1)
            for h in (2, 3):
                for i in range(KT // 2):
                    nc.sync.dma_start(out=wq_pairs[h][i][:], in_=wq[h, i])

            def load_wo(kt):
                # sync queue: mid-kernel it is idle, and gpsimd must stay
                # free for the rope-shift DMAs and collective triggers
                nc.sync.dma_start(out=wo_sb[kt][:], in_=wo[kt])

            # ---------------- persistent activations
            qt_sb = [apool.tile([D, S], BF, tag=f"qt{h}", name=f"qt{h}")
                     for h in range(G)]
            kt_sb = apool.tile([D, S], BF, tag="kt", name="kt")
            vtok_sb = apool.tile([128, TOKB, D], BF, tag="vtok", name="vtok")

            # ---------------- projections + RoPE + direct token-major v
            def rope_store(raw_ps, c, dst_slice):
                # copy raw to SBUF bf16 (frees the proj PSUM bank), then
                # u = raw*sin_pre, 64-partition shift of u via DMA, and
                # dst = shifted + raw*cos.  sin table pre-shifted on host.
                raw = work.tile([128, 512], BF, tag="rawb", name="rawb")
                nc.vector.tensor_copy(out=raw[:], in_=raw_ps)
                u = work.tile([128, 512], BF, tag="u", name="u")
                nc.vector.tensor_mul(u[:], raw[:], sin_t[c][:])
                sh = work.tile([128, 512], BF, tag="sh", name="sh")
                nc.gpsimd.dma_start(out=sh[0:64, :], in_=u[64:128, :])
                nc.gpsimd.dma_start(out=sh[64:128, :], in_=u[0:64, :])
                t2 = work.tile([128, 512], BF, tag="t2", name="t2")
                nc.vector.tensor_mul(t2[:], raw[:], cos_t[c][:])
                nc.vector.tensor_add(dst_slice, sh[:], t2[:])

            def proj_groups(c):
                """Chunk c's projection as 6 thunks (k, q0..q3, v)."""
                def g_k():
                    xc = xc_state[c]
                    ps = ps_pj.tile([128, 512], F32, tag="pj", name="pj")
                    for kt in range(KT):
                        nc.tensor.matmul(ps[:], wk_at(kt), xc[kt][:],
                                         start=(kt == 0), stop=(kt == KT - 1))
                    rope_store(ps[:], c, kt_sb[:, ds(512 * c, 512)])

                def mk_q(h):
                    def g_q():
                        xc = xc_state[c]
                        ps = ps_pj.tile([128, 512], F32, tag="pj", name="pj")
                        for kt in range(KT):
                            nc.tensor.matmul(ps[:], wq_at(h, kt),
                                             xc[kt][:],
                                             start=(kt == 0),
                                             stop=(kt == KT - 1))
                        rope_store(ps[:], c, qt_sb[h][:, ds(512 * c, 512)])
                    return g_q

                def g_v():
                    xc = xc_state[c]
                    ps = ps_pj.tile([128, 512], F32, tag="pj", name="pj")
                    for tb in range(4):
                        for kt in range(KT):
                            nc.tensor.matmul(ps[:, ts(tb, 128)],
                                             xc[kt][:, ts(tb, 128)],
                                             wv_at(kt),
                                             start=(kt == 0),
                                             stop=(kt == KT - 1))
                    nc.vector.tensor_copy(out=vtok_sb[:, ds(4 * c, 4), :],
                                          in_=ps[:])

                return {"k": g_k, "q0": mk_q(0), "q1": mk_q(1),
                        "q2": mk_q(2), "q3": mk_q(3), "v": g_v}

            # ---------------- collective buffers
            cin = [dpool.tile([D, G, 512], BF, tag=f"cin{t}", name=f"cin{t}")
                   for t in range(3)]
            cout = [dpool.tile([4, D, G, 512], BF, tag=f"cout{t}",
                               name=f"cout{t}") for t in range(3)]
            # quarter 3: heads 0/1 share one pair gather (issued after
            # head 1) so the CC core has fewer serialized collectives in
            # front of the tail-critical head-2/3 gathers
            cin3p = dpool.tile([D, 2, 512], BF, tag="cin3p", name="cin3p")
            cout3p = dpool.tile([4, D, 2, 512], BF, tag="cout3p",
                                name="cout3p")
            cin3 = [dpool.tile([D, 512], BF, tag=f"cin3{h}", name=f"cin3{h}")
                    for h in (2, 3)]
            cout3 = [dpool.tile([4, D, 512], BF, tag=f"cout3{h}",
                                name=f"cout3{h}") for h in (2, 3)]

            # ---------------- Wo: og loads + filler-granular matmuls
            og = {}

            def wo_loads(t):
                ogs = []
                for kt in range(KT):
                    r, h = divmod(kt, G)
                    o = ogpool.tile([128, 512], BF, tag="og", name="og")
                    nc.sync.dma_start(out=o[:], in_=cout[t][r, :, h, :])
                    ogs.append(o)
                og[t] = ogs

            pe_fill = deque()

            def queue_wo(t):
                """Enqueue quarter t's Wo work as single-matmul closures."""
                for tb in range(4):
                    state = {}

                    def mk(tb, pos, state):
                        def f():
                            if pos == 0:
                                state["pw"] = ps_wo.tile([128, 512], F32,
                                                         tag="wo", name="wo")
                            nc.tensor.matmul(state["pw"][:],
                                             og[t][pos][:, ts(tb, 128)],
                                             wo_sb[pos][:],
                                             start=(pos == 0),
                                             stop=(pos == KT - 1))
                            if pos == KT - 1:
                                ost = work.tile([128, 512], F32, tag="ost",
                                                name="ost", bufs=3)
                                nc.vector.tensor_copy(out=ost[:],
                                                      in_=state["pw"][:])
                                nc.sync.dma_start(
                                    out=out[ds(512 * t + 128 * tb, 128), :],
                                    in_=ost[:])
                        return f

                    for pos in range(KT):
                        pe_fill.append(mk(tb, pos, state))

            def pop_fill(n):
                for _ in range(n):
                    if not pe_fill:
                        return
                    pe_fill.popleft()()

            # ---------------- attention
            def emit_st(h, qc, kb):
                """score block, transposed: [k 128, q<=512] -> exp -> et.
                Causal mask accumulated via matmul (cmut.T @ iden); score
                scale is pre-folded into Wk on the host."""
                band = kb - 4 * qc
                et = etwork.tile([128, 512], BF, tag="et", name="et")
                sps = ps_sc.tile([128, 512], F32, tag="mm", name="mm")
                if band >= 0:
                    off = 128 * band
                    w = 512 - off
                    nc.tensor.matmul(sps[:, 0:128], cmut_sb[:], iden_sb[:],
                                     start=True, stop=False)
                    nc.tensor.matmul(sps[:, :w], kt_sb[:, ts(kb, 128)],
                                     qt_sb[h][:, ds(512 * qc + off, w)],
                                     start=False, stop=True)
                    if off:
                        nc.vector.memset(et[:, :off], 0.0)
                    nc.scalar.activation(
                        out=et[:, ds(off, w)], in_=sps[:, :w],
                        func=mybir.ActivationFunctionType.Exp)
                    return et, off
                nc.tensor.matmul(sps[:], kt_sb[:, ts(kb, 128)],
                                 qt_sb[h][:, ds(512 * qc, 512)],
                                 start=True, stop=True)
                nc.scalar.activation(
                    out=et[:], in_=sps[:],
                    func=mybir.ActivationFunctionType.Exp)
                return et, 0

            def emit_attn(qc, enq, pre, post):
                """enq: head -> [("loads", t) | ("wo", t) | ("wol", ...)]
                run at head start.  pre/post: head -> [proj thunks] at
                head start / head end."""
                nkb = 4 * qc + 4
                for h in range(G):
                    for act in enq.get(h, ()):
                        if act[0] == "loads":
                            wo_loads(act[1])
                        elif act[0] == "wo":
                            queue_wo(act[1])
                        elif act[0] == "xc":
                            load_xc(act[1])
                        else:
                            for kt in act[1]:
                                load_wo(kt)
                    for g in pre.get(h, ()):
                        g()
                    oT_ps = ps_pv.tile([128, 512], F32, tag="pv", name="pv")
                    den_ps = ps_den.tile([1, 512], F32, tag="den", name="den")
                    pend = [emit_st(h, qc, k)
                            for k in range(min(LOOKAHEAD, nkb))]
                    ngrp = (nkb + 3) // 4
                    esum = None
                    for kb in range(nkb):
                        if kb + LOOKAHEAD < nkb:
                            pend.append(emit_st(h, qc, kb + LOOKAHEAD))
                        et, off = pend.pop(0)
                        nc.tensor.matmul(oT_ps[:, ds(off, 512 - off)],
                                         vtok_sb[:, kb, :],
                                         et[:, ds(off, 512 - off)],
                                         start=(kb == 0), stop=(kb == nkb - 1))
                        # last head of quarter 3: hold fillers back for the
                        # tail, where the final gathers leave the PE idle
                        pop_fill(1 if (qc == 3 and h == 3) else 2)
                        # denominator: sum groups of 4 et tiles on DVE,
                        # then one ones-matmul per group
                        gi, gj = divmod(kb, 4)
                        last_in_grp = (gj == 3 or kb == nkb - 1)
                        if gj == 0:
                            esum = et
                        else:
                            nsum = etwork.tile([128, 512], BF, tag="esum",
                                               name="esum", bufs=3)
                            nc.vector.tensor_add(nsum[:], esum[:], et[:])
                            esum = nsum
                        if last_in_grp:
                            nc.tensor.matmul(den_ps[:], ones_sb[:, 0:1],
                                             esum[:],
                                             start=(gi == 0),
                                             stop=(gi == ngrp - 1))
                    rec = stats.tile([1, 512], F32, tag="recq", name="recq")
                    nc.vector.reciprocal_approx_fast(out=rec[:],
                                                     in_=den_ps[:])
                    bcast = bcpool.tile([128, 512], F32, tag="bcast",
                                        name="bcast")
                    nc.gpsimd.partition_broadcast(bcast[:], rec[:])
                    otst = work.tile([128, 512], BF, tag="otst", name="otst")
                    nc.vector.tensor_mul(otst[:], oT_ps[:], bcast[:])
                    if qc == 3:
                        if h < 2:
                            nc.gpsimd.dma_start(out=cin3p[:, h, :],
                                                in_=otst[:])
                            if h == 1:
                                nc.gpsimd.collective_compute(
                                    "AllGather", mybir.AluOpType.bypass,
                                    replica_groups=groups,
                                    ins=[cin3p.opt()], outs=[cout3p.opt()])
                        else:
                            nc.gpsimd.dma_start(out=cin3[h - 2][:],
                                                in_=otst[:])
                            nc.gpsimd.collective_compute(
                                "AllGather", mybir.AluOpType.bypass,
                                replica_groups=groups,
                                ins=[cin3[h - 2].opt()],
                                outs=[cout3[h - 2].opt()])
                    else:
                        nc.gpsimd.dma_start(out=cin[qc][:, h, :], in_=otst[:])
                    pop_fill(4)
                    for g in post.get(h, ()):
                        g()
                if qc != 3:
                    nc.gpsimd.collective_compute(
                        "AllGather", mybir.AluOpType.bypass,
                        replica_groups=groups,
                        ins=[cin[qc].opt()], outs=[cout[qc].opt()])

            # ---------------- schedule
            pg = {c: proj_groups(c) for c in range(TCH)}
            # chunk-0 projections in DMA-arrival order; q1..q3 fold into
            # quarter 0 one head early so rope latency hides under the
            # previous head's attention.
            pg[0]["k"]()
            pg[0]["q0"]()
            pg[0]["v"]()
            emit_attn(0, {1: [("xc", 2)], 3: [("xc", 3)]},
                      {0: [pg[0]["q1"]], 1: [pg[0]["q2"]],
                       2: [pg[0]["q3"]], 3: [pg[1]["v"]]},
                      {0: [pg[1]["k"]], 1: [pg[1]["q0"]],
                       2: [pg[1]["q1"]], 3: [pg[1]["q2"]]})
            # wo(t) matmuls are consumed one full quarter after gather(t)
            # is issued so they never stall on collective latency
            emit_attn(1, {0: [("loads", 0), ("wol", range(0, 8))],
                          1: [("wol", range(8, 16))]},
                      {0: [pg[1]["q3"]], 2: [pg[2]["q1"]],
                       3: [pg[2]["v"]]},
                      {0: [pg[2]["k"]], 1: [pg[2]["q0"]],
                       2: [pg[2]["q2"]], 3: [pg[2]["q3"]]})
            emit_attn(2, {0: [("loads", 1), ("wo", 0)]},
                      {1: [pg[3]["q0"]], 2: [pg[3]["q2"]],
                       3: [pg[3]["v"]]},
                      {0: [pg[3]["k"]], 1: [pg[3]["q1"]],
                       2: [pg[3]["q3"]], 3: []})
            emit_attn(3, {0: [("loads", 2), ("wo", 1)], 3: [("wo", 2)]},
                      {}, {})

            # ---------------- tail: four Wo waves, one per gathered head;
            # remaining wo(2) fillers interleave between the waves so the
            # PE has work while each head's gather is in flight
            pw = [None] * 4
            for h in range(G):
                ogs = []
                for r in range(4):
                    o = ogpool.tile([128, 512], BF, tag="og", name="og")
                    if h < 2:
                        nc.sync.dma_start(out=o[:], in_=cout3p[r, :, h, :])
                    else:
                        nc.sync.dma_start(out=o[:],
                                          in_=cout3[h - 2][r, :, :])
                    ogs.append(o)
                # waves 0/1's gathers are long done by the time the PE
                # gets here; spend the fillers where waves 2/3 wait
                pop_fill(0 if h < 2 else 22)
                for tb in range(4):
                    if h == 0:
                        # ps_wo stays reserved for the interleaved wo(2)
                        # fillers - sharing it here would deadlock the ring
                        pool = [ps_pj, ps_pv, ps_pv, ps_sc][tb]
                        tag = ["pj", "pv", "pv", "mm"][tb]
                        pw[tb] = pool.tile([128, 512], F32, tag=tag,
                                           name="pwt")
                    for r in range(4):
                        nc.tensor.matmul(pw[tb][:],
                                         ogs[r][:, ts(tb, 128)],
                                         wo_sb[r * G + h][:],
                                         start=(h == 0 and r == 0),
                                         stop=(h == G - 1 and r == 3))
                    if h == G - 1:
                        ost = work.tile([128, 512], F32, tag="ost",
                                        name="ost", bufs=3)
                        nc.vector.tensor_copy(out=ost[:], in_=pw[tb][:])
                        nc.sync.dma_start(
                            out=out[ds(512 * 3 + 128 * tb, 128), :],
                            in_=ost[:])
            pop_fill(len(pe_fill))

    nc.finalize()
    return nc


_NC_CACHE = {}


def _get_nc():
    if "nc" not in _NC_CACHE:
        _NC_CACHE["nc"] = build_nc()
    return _NC_CACHE["nc"]


def _rope_tables():
    inv = 1.0 / (THETA ** (np.arange(0, D, 2, dtype=np.float64) / D))  # [64]
    pos = np.arange(S, dtype=np.float64)
    fr = pos[:, None] * inv[None, :]                 # [S, 64]
    emb = np.concatenate([fr, fr], axis=1)           # [S, D]
    cos = np.cos(emb).T.astype(np.float32)           # [D, S]
    sin = np.sin(emb).T.astype(np.float32)
    sgn = np.where(np.arange(D) < D // 2, -1.0, 1.0).astype(np.float32)[:, None]
    sink = sin * sgn                                 # sign-folded sin
    # pre-shift by 64 partitions: u[p] = raw[p]*sink[(p+64)%128] then a
    # 64-partition rotation of u gives rotate_half(raw)*sink exactly
    sink_pre = np.roll(sink, 64, axis=0)
    # per-chunk bf16 tiles [TCH, D, 512]
    cos_t = np.ascontiguousarray(
        cos.reshape(D, TCH, 512).transpose(1, 0, 2)).astype(BF16)
    sin_t = np.ascontiguousarray(
        sink_pre.reshape(D, TCH, 512).transpose(1, 0, 2)).astype(BF16)
    return cos_t, sin_t


def kernel(x, Wq, Wk, Wv, Wo):
    global LAST_EXEC_TIME_NS, LAST_RESULTS
    nc = _get_nc()
    ct, st = _rope_tables()
    cmut_np = np.where(np.arange(128)[None, :] > np.arange(128)[:, None],
                       np.float32(NEG), np.float32(0.0)).astype(BF16)
    iden_np = np.eye(128, dtype=np.float32).astype(BF16)
    scale = np.float32(D ** -0.5)
    in_maps = []
    for c in range(N_CORES):
        b, kh = c // 4, c % 4
        xT = np.ascontiguousarray(x[b].T).astype(BF16)       # [DM, S]
        xtile = np.ascontiguousarray(
            xT.reshape(KT, 128, TCH, 512).transpose(2, 0, 1, 3))
        def pair(a):  # [KT, 128, W] -> [KT//2, 128, 2W]
            kt, p, w = a.shape
            return np.ascontiguousarray(
                a.reshape(kt // 2, 2, p, w).transpose(0, 2, 1, 3)
                .reshape(kt // 2, p, 2 * w))

        wq_s = Wq[:, kh * HD_CORE:(kh + 1) * HD_CORE]        # [DM, 512]
        wq_t = np.ascontiguousarray(
            np.stack([pair(np.ascontiguousarray(
                wq_s.reshape(KT, 128, G, 128)[:, :, h, :]))
                for h in range(G)])).astype(BF16)
        wk_t = pair(np.ascontiguousarray(
            (Wk[:, kh * D:(kh + 1) * D] * scale).reshape(KT, 128, D))
        ).astype(BF16)
        wv_t = pair(np.ascontiguousarray(
            Wv[:, kh * D:(kh + 1) * D].reshape(KT, 128, D))).astype(BF16)
        wo_t = np.ascontiguousarray(
            Wo[:, kh * HD_CORE:(kh + 1) * HD_CORE].reshape(KT, 128, HD_CORE)
        ).astype(BF16)
        in_maps.append({
            "xt": xtile, "wq": wq_t, "wk": wk_t, "wv": wv_t, "wo": wo_t,
            "cosb": ct, "sinb": st, "cmut": cmut_np, "iden": iden_np,
        })
    trace = os.environ.get("KERNEL_TRACE", "0") == "1" and _install_ntff_hook()
    if os.environ.get("KERNEL_WARMUP", "1") == "1":
        # Untraced warm-up execution: first-launch NEFF load/JIT skews the 8
        # cores by 10-100us, which lands in core 0's collective waits.
        run_bass_kernel_spmd(nc, in_maps, core_ids=list(range(N_CORES)),
                             trace=False)
    res = run_bass_kernel_spmd(nc, in_maps, core_ids=list(range(N_CORES)),
                               trace=trace)
    LAST_EXEC_TIME_NS = res.exec_time_ns
    LAST_RESULTS = res
    out = np.empty((B, S, DM), dtype=np.float32)
    for c in range(N_CORES):
        b, kh = c // 4, c % 4
        out[b, :, kh * HD_CORE:(kh + 1) * HD_CORE] = res.results[c]["out"]
    return out
